# revision 29
# baseline (speedup 1.0000x reference)
"""Trainium2 Bass kernel for nn_BitfieldLinear (vq_codebook).

Reference computation:
    idx   = codes & 0xFF            (basis row, 256 entries)
    r_q   = (codes >> 8) & 0xFFF
    sign  = bit20 ? -1 : +1
    scale = sign * tanh(r_q / 4095)
    W     = scale[:, None] * basis[idx]        # [8192, 4096]
    y     = x @ W.T                            # [128, 8192]

Key factorization (never materialize the 128MB W):
    Z = x @ basis.T                            # [128, 256]  tiny matmul
    y[b, j] = scale[j] * Z[b, idx[j]]          # column gather + scale

Sharding: outputs are binned BY BASIS INDEX — core c owns every output j
with idx[j] // 32 == c (a data-dependent column permutation, undone on
the host).  Core c then only needs basis rows [32c, 32c+32): its Z block
Z[:, 32c:32c+32] = x @ basis[32c:32c+32].T is computed locally from the
full x (1MB fp16) + a 256KB basis shard — ~1.3MB/core vs 3.25MB for the
replicate-everything layout, no cross-core traffic.

Per core:
    1. stream everything fp16 (codes padded to full 512B partition
       lines lead the sync ring; the scalar ring carries the two tail
       x chunks); dummy matmuls warm the PE's HAM clock gate to 2.4GHz
    2. 32 matmuls accumulate Z^T [32o, 128b] directly (lhsT = 32-col
       basis tile — no transpose needed before the y matmul)
    3. decode on-chip: DVE bitops + odd-poly tanh (scale = sign*P(r),
       sign folded in last); G[k, j] = scale[j]*(idx[j]-32c == k) built
       in ONE is_equal over broadcast-replicated idx/scale rows, then
       PE-transposed tile-wise into a wide PSUM strip -> G [32, 1152]
    4. y = Z^T.T @ G in 3 PSUM column chunks; casts split across
       vector/scalar; two parallel fp16 stores
Host reassembles: out[:, bin_cols_c] = y_c[:, :n_c] (pure scatter).
Overall rel err ~7e-4 (fp16 rounding), vs 2e-2 tolerance.
"""

import sys

for _p in ("/opt/trn_rl_repo", "/opt/pypackages"):
    if _p not in sys.path:
        sys.path.insert(0, _p)

import numpy as np

import concourse.bacc as bacc
import concourse.mybir as mybir
import concourse.tile as tile
from concourse.alu_op_type import AluOpType
from concourse.bass_utils import run_bass_kernel_spmd

N_CORES = 8
BATCH = 128
IN_F = 4096
OUT_F = 8192
BASIS = 256
ROWS = BASIS // N_CORES     # 32 basis rows per core
OPC = 1152                  # padded outputs per core (max bin 1088)
NT = OPC // 128             # 10 code-tiles per core
NK = IN_F // 128            # 32 K-tiles
R_LEVELS = 4095.0

# odd polynomial tanh(s) ~ s*(C0 + C1 s^2 + C2 s^4) on [0, 1];
# max abs err 8.7e-4 (well under the fp16 noise floor here)
C0, C1, C2 = 0.99766471, -0.31069468, 0.07549324

F32 = mybir.dt.float32
FP16 = mybir.dt.float16
I32 = mybir.dt.int32

# x k-tile ranges per DMA chunk: m0 also carries ident+basis.  The
# sync (qSP) ring gets more SDMA bandwidth than qAct, so it carries
# ~60%; the scalar ring's two tail chunks have slack, so the hoisted
# ACT-table load on that engine is harmless.
X_RANGES = [(0, 9), (9, 18), (18, 26), (26, 32)]
CHUNK_ENG = ["sync", "sync", "scalar", "scalar"]

Y_CHUNKS = [(0, 512), (512, 512), (1024, 128)]


def build_nc():
    nc = bacc.Bacc(
        "TRN2",
        target_bir_lowering=False,
        debug=False,
        num_devices=N_CORES,
    )

    # codes padded to full 512-byte partition lines: 44B lines made the
    # DMA's descriptors pathologically slow to process
    c128_d = nc.dram_tensor("c128", [128, 128], I32, kind="ExternalInput")
    m_ds = []
    for i, (ks, ke) in enumerate(X_RANGES):
        w = (ke - ks) * 128 + (128 + NK * ROWS if i == 0 else 0)
        m_ds.append(
            nc.dram_tensor(f"m{i}", [128, w], FP16, kind="ExternalInput")
        )
    out_d = nc.dram_tensor("out", [128, OPC], FP16, kind="ExternalOutput")

    with tile.TileContext(nc) as tc:
        with (
            tc.tile_pool(name="pool", bufs=1) as pool,
            tc.tile_pool(name="zps", bufs=1, space="PSUM") as zps,
            tc.tile_pool(name="yps", bufs=1, space="PSUM") as yps,
        ):
            # ---- DMA issue: codes tensor first (unblocks decode), then
            # the bulk fp16 chunks, all on the sync ring
            c128 = pool.tile([128, 128], I32)
            nc.sync.dma_start(out=c128[:], in_=c128_d[:])
            m_sb = []
            for i, (ks, ke) in enumerate(X_RANGES):
                t = pool.tile(list(m_ds[i].shape), FP16, name=f"m_sb{i}")
                m_sb.append(t)
                eng = nc.sync if CHUNK_ENG[i] == "sync" else nc.scalar
                eng.dma_start(out=t[:], in_=m_ds[i][:])

            ident16 = m_sb[0][:, :128]

            def xtile(n):
                for i, (ks, ke) in enumerate(X_RANGES):
                    if ks <= n < ke:
                        off = (n - ks) * 128 + (128 + NK * ROWS if i == 0 else 0)
                        return m_sb[i][:, off : off + 128]
                raise AssertionError(n)

            def btile(n):
                return m_sb[0][:, 128 + n * ROWS : 128 + (n + 1) * ROWS]

            # ---- PE warm-up: dummy matmuls on a memset scratch during
            # the pre-stream idle so the HAM clock gate ramps the PE to
            # 2.4 GHz; they target z's PSUM bank, which the first real
            # accumulation (start=True) then reclaims
            z_ps = zps.tile([ROWS, 128], F32, tag="z")
            warm_sb = pool.tile([128, 128], FP16, name="warm_sb")
            nc.gpsimd.memset(warm_sb[:], 0.0)
            for wi in range(40):
                nc.tensor.matmul(
                    z_ps[:], lhsT=warm_sb[:, :ROWS], rhs=warm_sb[:],
                    start=True, stop=True,
                )

            # ---- iota row 0..31 (small ops on gpsimd, done before the
            # DVE gets busy), then replicated NT times on DVE via a
            # stride-0 broadcast read.  GpSimd is ~15x slower at wide
            # ops and its SBUF traffic stalls the DVE, so keep it tiny.
            iota_i = pool.tile([128, ROWS], I32)
            nc.gpsimd.iota(
                iota_i[:], pattern=[[1, ROWS]], channel_multiplier=0
            )
            iota32 = pool.tile([128, ROWS], F32)
            nc.gpsimd.tensor_scalar_mul(
                out=iota32[:], in0=iota_i[:], scalar1=1.0
            )
            iota_rep = pool.tile([128, NT * ROWS], F32)
            nc.vector.tensor_copy(
                out=iota_rep[:],
                in_=iota32[:, None, :].to_broadcast([128, NT, ROWS]),
            )

            # ---- decode (all DVE; ~11-op serial chain after the codes
            # land).  scale = sign * P(r) with sign folded in last, so
            # the poly chain runs on |r| directly.
            def emit_decode():
                idx_i = pool.tile([128, NT], I32, name="idx_i")
                nc.vector.tensor_scalar(
                    out=idx_i[:], in0=c128[:, :NT],
                    scalar1=255, scalar2=None, op0=AluOpType.bitwise_and,
                )
                idx_f0 = pool.tile([128, NT], F32, name="idx_f0")
                nc.vector.tensor_scalar_mul(
                    out=idx_f0[:], in0=idx_i[:], scalar1=1.0
                )
                base_f = pool.tile([128, 1], F32, name="base_f")
                nc.vector.tensor_scalar_mul(
                    out=base_f[:], in0=c128[:, NT : NT + 1], scalar1=1.0
                )
                idx_f = pool.tile([128, NT], F32, name="idx_f")
                nc.vector.tensor_scalar(
                    out=idx_f[:], in0=idx_f0[:],
                    scalar1=base_f[:], scalar2=None, op0=AluOpType.subtract,
                )
                # replicate idx across the 32 one-hot columns via a
                # stride-0 broadcast read
                idx_rep = pool.tile([128, NT * ROWS], F32, name="idx_rep")
                nc.vector.tensor_copy(
                    out=idx_rep[:],
                    in_=idx_f[:, :, None].to_broadcast([128, NT, ROWS]),
                )
                eq_all = pool.tile([128, NT * ROWS], F32, name="eq_all")
                nc.vector.tensor_tensor(
                    out=eq_all[:], in0=idx_rep[:], in1=iota_rep[:],
                    op=AluOpType.is_equal,
                )

                rq_i = pool.tile([128, NT], I32, name="rq_i")
                nc.vector.tensor_scalar(
                    out=rq_i[:], in0=c128[:, :NT],
                    scalar1=8, scalar2=4095,
                    op0=AluOpType.logical_shift_right,
                    op1=AluOpType.bitwise_and,
                )
                sg_i = pool.tile([128, NT], I32, name="sg_i")
                nc.vector.tensor_scalar(
                    out=sg_i[:], in0=c128[:, :NT],
                    scalar1=20, scalar2=1,
                    op0=AluOpType.logical_shift_right,
                    op1=AluOpType.bitwise_and,
                )
                pm = pool.tile([128, NT], F32, name="pm")
                nc.vector.tensor_scalar(
                    out=pm[:], in0=sg_i[:],
                    scalar1=-2.0, scalar2=1.0,
                    op0=AluOpType.mult, op1=AluOpType.add,
                )
                sr0 = pool.tile([128, NT], F32, name="sr0")
                nc.vector.tensor_scalar_mul(
                    out=sr0[:], in0=rq_i[:], scalar1=1.0 / R_LEVELS
                )
                t2 = pool.tile([128, NT], F32, name="t2")
                nc.vector.tensor_tensor(
                    out=t2[:], in0=sr0[:], in1=sr0[:], op=AluOpType.mult
                )
                u = pool.tile([128, NT], F32, name="u")
                nc.vector.tensor_scalar(
                    out=u[:], in0=t2[:], scalar1=C2, scalar2=C1,
                    op0=AluOpType.mult, op1=AluOpType.add,
                )
                nc.vector.tensor_tensor(
                    out=u[:], in0=u[:], in1=t2[:], op=AluOpType.mult
                )
                nc.vector.tensor_scalar(
                    out=u[:], in0=u[:], scalar1=1.0, scalar2=C0,
                    op0=AluOpType.mult, op1=AluOpType.add,
                )
                s1 = pool.tile([128, NT], F32, name="s1")
                nc.vector.tensor_tensor(
                    out=s1[:], in0=u[:], in1=sr0[:], op=AluOpType.mult
                )
                scl = pool.tile([128, NT], F32, name="scl")
                nc.vector.tensor_tensor(
                    out=scl[:], in0=s1[:], in1=pm[:], op=AluOpType.mult
                )
                scl_rep = pool.tile([128, NT * ROWS], F32, name="scl_rep")
                nc.vector.tensor_copy(
                    out=scl_rep[:],
                    in_=scl[:, :, None].to_broadcast([128, NT, ROWS]),
                )
                # gt_all[p, 32t + k] = scl[t*128+p] * (idx[t*128+p] == k)
                nc.vector.tensor_tensor(
                    out=gt_all[:], in0=eq_all[:], in1=scl_rep[:],
                    op=AluOpType.mult,
                )

            gt_all = pool.tile([128, NT * ROWS], FP16, name="gt_all")

            # ---- G [32, OPC]: PE-transpose each [128, 32] gt slice into
            # a wide PSUM strip, then two bulk copies to SBUF
            tp_wide = yps.tile([32, OPC], FP16, tag="tpw", name="tp_wide")
            g16 = pool.tile([32, OPC], FP16)

            def emit_g_transpose(t):
                nc.tensor.transpose(
                    out=tp_wide[:, t * 128 : (t + 1) * 128],
                    in_=gt_all[:, t * ROWS : (t + 1) * ROWS],
                    identity=ident16,
                )

            G_COPY_SPLIT = [(0, 512), (512, 640)]

            def emit_g_copy(h):
                off, w = G_COPY_SPLIT[h]
                if h == 0:
                    nc.vector.tensor_copy(
                        out=g16[:, off : off + w],
                        in_=tp_wide[:, off : off + w],
                    )
                else:
                    nc.scalar.copy(
                        out=g16[:, off : off + w],
                        in_=tp_wide[:, off : off + w],
                    )

            # ---- Z^T [32o, 128b] accumulated over all 32 k-tiles;
            # the G transposes/copies are emitted early so they run in
            # the PE's stream gaps as soon as gt_all lands
            for ci, (ks, ke) in enumerate(X_RANGES):
                for n in range(ks, ke):
                    nc.tensor.matmul(
                        z_ps[:],
                        lhsT=btile(n),
                        rhs=xtile(n),
                        start=(n == 0), stop=(n == NK - 1),
                    )
                if ci == 0:
                    emit_decode()
                    for t in range(NT):
                        emit_g_transpose(t)
                    emit_g_copy(0)
                    emit_g_copy(1)

            zt16 = pool.tile([32, 128], FP16, name="zt16")
            nc.vector.tensor_copy(out=zt16[:], in_=z_ps[:])

            # ---- y = Z^T.T @ G in 3 column chunks; cast+store each as
            # it completes (casts split vector/scalar, stores split
            # across both rings)
            y16 = pool.tile([128, OPC], FP16)
            for q, (off, w) in enumerate(Y_CHUNKS):
                y_ps = yps.tile([128, w], F32, tag=f"y{q}", name=f"y_ps{q}")
                nc.tensor.matmul(
                    y_ps[:],
                    lhsT=zt16[:],
                    rhs=g16[:, off : off + w],
                    start=True, stop=True,
                )
                if q == 1:
                    nc.scalar.copy(out=y16[:, off : off + w], in_=y_ps[:])
                else:
                    nc.vector.tensor_copy(
                        out=y16[:, off : off + w], in_=y_ps[:]
                    )
            nc.sync.dma_start(out=out_d[:, :512], in_=y16[:, :512])
            nc.scalar.dma_start(out=out_d[:, 512:], in_=y16[:, 512:])

    nc.compile()
    return nc


_NC = None


def _get_nc():
    global _NC
    if _NC is None:
        _NC = build_nc()
    return _NC


def make_in_maps(x, codes, basis):
    x = np.ascontiguousarray(x, dtype=np.float32)
    basis = np.ascontiguousarray(basis, dtype=np.float32)
    codes = np.ascontiguousarray(codes, dtype=np.int32)

    # xt[p, n*128 + m] = x[m, n*128 + p]  (shared across cores)
    xt = np.ascontiguousarray(
        x.reshape(BATCH, NK, 128).transpose(2, 1, 0).reshape(128, IN_F)
    ).astype(np.float16)
    xslices = [xt[:, ks * 128 : ke * 128] for ks, ke in X_RANGES]
    ident = np.eye(128, dtype=np.float16)

    idx_all = codes & 255
    bins = idx_all // ROWS

    in_maps = []
    sels = []
    for c in range(N_CORES):
        sel = np.where(bins == c)[0]
        assert len(sel) <= OPC, f"core {c} bin overflow: {len(sel)}"
        sels.append(sel)
        padded = np.zeros(OPC, dtype=np.int32)
        padded[: len(sel)] = codes[sel]
        c128 = np.zeros((128, 128), dtype=np.int32)
        c128[:, :NT] = padded.reshape(NT, 128).T
        c128[:, NT] = c * ROWS

        # bt[p, n*32 + o] = basis[32c + o, n*128 + p]
        bt = np.ascontiguousarray(
            basis[c * ROWS : (c + 1) * ROWS]
            .reshape(ROWS, NK, 128)
            .transpose(2, 1, 0)
            .reshape(128, NK * ROWS)
        ).astype(np.float16)

        im = {"c128": c128}
        for i in range(len(X_RANGES)):
            if i == 0:
                im["m0"] = np.ascontiguousarray(
                    np.concatenate([ident, bt, xslices[0]], axis=1)
                )
            else:
                im[f"m{i}"] = np.ascontiguousarray(xslices[i])
        in_maps.append(im)
    return in_maps, sels


def assemble_output(results, sels):
    out = np.zeros((BATCH, OUT_F), dtype=np.float32)
    for c in range(N_CORES):
        sel = sels[c]
        out[:, sel] = results[c]["out"][:, : len(sel)].astype(np.float32)
    return out


def kernel(x, codes, basis):
    nc = _get_nc()
    in_maps, sels = make_in_maps(x, codes, basis)
    res = run_bass_kernel_spmd(nc, in_maps, list(range(N_CORES)))
    return assemble_output(res.results, sels)


if __name__ == "__main__":
    rng = np.random.default_rng(0)
    x = rng.standard_normal((BATCH, IN_F), dtype=np.float32)
    basis = (rng.standard_normal((BASIS, IN_F)) * 0.02).astype(np.float32)
    codes = rng.integers(0, 1 << 22, size=(OUT_F,), dtype=np.int32)
    y = kernel(x, codes, basis)

    idx = codes & 255
    r = ((codes >> 8) & 4095).astype(np.float32) / R_LEVELS
    sign = np.where(((codes >> 20) & 1) == 1, -1.0, 1.0).astype(np.float32)
    scale = sign * np.tanh(r)
    W = scale[:, None] * basis[idx]
    y_ref = x @ W.T
    err = np.linalg.norm(y - y_ref) / np.linalg.norm(y_ref)
    print("rel err:", err)
